# revision 4
# baseline (speedup 1.0000x reference)
"""Trainium2 Bass kernel for nn_FAM (dynamic grouped 3x3 low-pass filter + frequency gating).

Data-parallel over batch: 16 images -> 8 cores x 2 images.

v6: bf16 streaming design. Host reflect-pads columns, casts x to bf16 and
reorders to a DMA-contiguous layout [n][seg of 32ch][h=128 part][32c x 132w];
output returns as bf16 [n][seg][h][32c x 128w] and is upcast/reordered on
host. Device DMA is ~34 MB/core of large contiguous transfers (vs 67 MB f32).

Per-core algorithm (per image):
  at load, per 32-ch segment:
    xs1[c] = (s1/s2)[c] * x[c]      (DVE tensor_scalar bf16 4x, accum_out)
    edge[h,c] = x[h,c,2]+x[h,c,127] (DVE; reflect-pad overcount)
  pooled[c] = (sum_h accum)[c]*(s2/s1)[c] - (sum_h edge)[c]   (PE ones-MM + DVE)
  filt = tanh(BN(conv_w @ pooled))  (PE + ACT tanh)
  G_dx = sum_dy filt[g,dy*3+dx]*D_dy  (ACT scale + DVE adds -> bf16)
  per segment, 4-ch matmul batches (N=512), 3-q PSUM waves:
    PSUM = I^T @ x_raw + sum_dx G_dx^T @ xs1_dxview   = (s1/s2)*low + x
    outst = copy(PSUM)              (per-q plain copy, split DVE/ACT)
    outst[c] = s2[c]*outst[c] + beta[n,c]   (GPSIMD in-place bf16)
where s1 = (ia+1)(ll+1)-(lh+1), s2 = lh+1, beta = -ia*(ll+1)*mean(x[c]).
"""

import os
import sys

for _p in ("/opt/trn_rl_repo", "/opt/pypackages"):
    if _p not in sys.path and os.path.isdir(_p):
        sys.path.append(_p)

from contextlib import ExitStack

import numpy as np
import ml_dtypes

import concourse.bass as bass
import concourse.tile as tile
from concourse import bacc, mybir
from concourse.bass_utils import run_bass_kernel_spmd

F32 = mybir.dt.float32
BF16 = mybir.dt.bfloat16
AF = mybir.ActivationFunctionType
ALU = mybir.AluOpType
NPBF16 = ml_dtypes.bfloat16

N_CORES = 8
N_PER_CORE = 2        # images per core
C = 256               # channels
G = 8                 # groups
H = W = 128
HW = H * W
K = 3
BN_EPS = 1e-5
SEG_CH = 32           # channels per segment (= one group)
N_SEG = C // SEG_CH   # 8 segments per image
WPAD = 132            # per-channel row stride (130 used + 2 dead, 4B-aligned)
WUSE = 130            # reflect-padded row: cols 0..129
BATCH_CH = 4          # channels per matmul batch (N = 4*128 = 512)
WAVE_Q = 3            # q-batches per PSUM wave (3 banks, double-buffered in 6)
SEG_BUFS = 9          # raw-x ring slots
XS1_BUFS = 9          # xs1 ring slots


def _reflect(i: int) -> int:
    if i < 0:
        return -i
    if i > H - 1:
        return 2 * (H - 1) - i
    return i


def _host_consts(conv_w, bn_gamma, bn_beta, bn_mean, bn_var, lamb_l, lamb_h, inside_all):
    """Host-side parameter prep (no x-dependent math)."""
    s_bn = bn_gamma / np.sqrt(bn_var + BN_EPS)
    bn_scale = (s_bn / HW).astype(np.float32)
    bn_bias = (bn_beta - bn_mean * s_bn).astype(np.float32)
    bnsb = np.stack([bn_scale, bn_bias], axis=1)          # [72, 2]

    s1 = (inside_all + 1.0) * (lamb_l + 1.0) - (lamb_h + 1.0)
    s2 = lamb_h + 1.0
    mb = -inside_all * (lamb_l + 1.0) / HW
    sbc = np.concatenate([s1 / s2, s2]).astype(np.float32)  # [512]
    sbc = np.broadcast_to(sbc[None, :], (128, 512)).copy()  # [128, 512]
    ivr = (s2 / s1).astype(np.float32).reshape(1, 256).copy()
    mbrow = mb.astype(np.float32).reshape(1, 256).copy()    # [1, 256]

    d_up = np.zeros((128, 128), np.float32)
    d_dn = np.zeros((128, 128), np.float32)
    idn = np.eye(128, dtype=np.float32)
    for h in range(H):
        d_up[_reflect(h - 1), h] = 1.0
        d_dn[_reflect(h + 1), h] = 1.0
    dmats = np.concatenate([d_up, idn, d_dn], axis=1)     # [128, 384]
    idnb = idn.astype(NPBF16)                             # [128, 128] bf16

    wt = conv_w.T.astype(np.float32)                      # [256, 72]
    wtd = np.concatenate([wt[:128], wt[128:]], axis=1)    # [128, 144]

    return dict(dmats=dmats, sbc=sbc, ivr=ivr, mbrow=mbrow, wtd=wtd,
                bnsb=bnsb, idnb=idnb)


def _host_pack_x(x):
    """[16, 256, 128, 128] f32 -> [16, 8, 128, 32*132] bf16, reflect-padded."""
    xp = np.pad(x, ((0, 0), (0, 0), (0, 0), (1, 1)), mode="reflect")
    xp = xp.astype(NPBF16)                                # [16, 256, 128, 130]
    xr = xp.reshape(16, N_SEG, SEG_CH, H, WUSE).transpose(0, 1, 3, 2, 4)
    packed = np.zeros((16, N_SEG, H, SEG_CH, WPAD), NPBF16)
    packed[..., :WUSE] = xr
    return np.ascontiguousarray(packed.reshape(16, N_SEG, H, SEG_CH * WPAD))


def _host_unpack_out(res_outs):
    """8 x [2, 8, 128, 32*128] bf16 -> [16, 256, 128, 128] f32."""
    out = np.empty((16, C, H, W), np.float32)
    for i, o in enumerate(res_outs):
        o = np.asarray(o).reshape(N_PER_CORE, N_SEG, H, SEG_CH, W)
        o = o.transpose(0, 1, 3, 2, 4).astype(np.float32)
        out[i * N_PER_CORE:(i + 1) * N_PER_CORE] = o.reshape(
            N_PER_CORE, C, H, W)
    return out


def _build_kernel(ctx: ExitStack, tc: "tile.TileContext",
                  x_ap: bass.AP, out_ap: bass.AP,
                  dmats_ap: bass.AP, sbc_ap: bass.AP, ivr_ap: bass.AP,
                  mbrow_ap: bass.AP, wtd_ap: bass.AP, bnsb_ap: bass.AP,
                  idnb_ap: bass.AP):
    nc = tc.nc

    cpool = ctx.enter_context(tc.tile_pool(name="consts", bufs=1))
    stpool = ctx.enter_context(tc.tile_pool(name="stats", bufs=1))
    segpool = ctx.enter_context(tc.tile_pool(name="seg", bufs=SEG_BUFS))
    xspool = ctx.enter_context(tc.tile_pool(name="xs1", bufs=XS1_BUFS))
    opool = ctx.enter_context(tc.tile_pool(name="outst", bufs=3))
    mpsum = ctx.enter_context(tc.tile_pool(name="mpsum", bufs=6, space="PSUM"))
    spsum = ctx.enter_context(tc.tile_pool(name="spsum", bufs=2, space="PSUM"))

    # ---- constants to SBUF ----
    dmats_sb = cpool.tile([128, 384], F32)
    nc.sync.dma_start(dmats_sb[:], dmats_ap)
    sbc_sb = cpool.tile([128, 512], F32)
    nc.sync.dma_start(sbc_sb[:], sbc_ap)
    ivr_sb = cpool.tile([1, 256], F32)
    nc.sync.dma_start(ivr_sb[:], ivr_ap)
    mbrow_sb = cpool.tile([1, 256], F32)
    nc.sync.dma_start(mbrow_sb[:], mbrow_ap)
    wtd_sb = cpool.tile([128, 144], F32)
    nc.sync.dma_start(wtd_sb[:], wtd_ap)
    bnsb_sb = cpool.tile([72, 2], F32)
    nc.sync.dma_start(bnsb_sb[:], bnsb_ap)
    idnb_sb = cpool.tile([128, 128], BF16)
    nc.sync.dma_start(idnb_sb[:], idnb_ap)
    ones_sb = cpool.tile([1, 128], F32)
    nc.vector.memset(ones_sb[:], 1.0)
    onescol = cpool.tile([128, 1], F32)
    nc.vector.memset(onescol[:], 1.0)

    idn = dmats_sb[:, 128:256]                            # [128,128] identity

    # persistent per-image tiles
    racc, redge, fbs, b_n, gt = {}, {}, {}, {}, {}
    for n in range(N_PER_CORE):
        racc[n] = stpool.tile([128, 256], F32, name=f"racc_{n}")
        redge[n] = stpool.tile([128, 256], F32, name=f"redge_{n}")
        fbs[n] = stpool.tile([128, 72], F32, name=f"fbs_{n}")
        b_n[n] = stpool.tile([128, 256], F32, name=f"bn_{n}")
        gt[n] = stpool.tile([128, G * 3 * 128], BF16, name=f"gt_{n}")

    segs = {}   # (n, s) -> raw seg tile
    xs1s = {}   # (n, s) -> xs1 tile

    def load_seg(n, s):
        """DMA one 32-ch segment; edge-fix sums; xs1 prescale with accum."""
        c0 = s * SEG_CH
        seg = segpool.tile([128, SEG_CH * WPAD], BF16, name="seg", tag="seg")
        segs[(n, s)] = seg
        nc.sync.dma_start(seg[:], x_ap[n, s])
        s3 = seg.rearrange("p (c w) -> p c w", c=SEG_CH)
        nc.vector.tensor_tensor(
            redge[n][:, c0:c0 + SEG_CH], s3[:, :, 2], s3[:, :, 127],
            op=ALU.add)
        xs1 = xspool.tile([128, SEG_CH * WPAD], BF16, name="xs1", tag="xs1")
        xs1s[(n, s)] = xs1
        xs13 = xs1.rearrange("p (c w) -> p c w", c=SEG_CH)
        for ci in range(SEG_CH):
            c = c0 + ci
            nc.vector.tensor_scalar(
                out=xs13[:, ci, 0:WUSE], in0=s3[:, ci, 0:WUSE],
                scalar1=sbc_sb[:, c:c + 1], scalar2=None, op0=ALU.mult,
                op1=ALU.add, accum_out=racc[n][:, c:c + 1])

    def filt_branch(n):
        # pooled_row[1, c] = (sum_h racc)*ivr - sum_h redge
        prp = spsum.tile([1, 256], F32, name="prp", tag="sp")
        nc.tensor.matmul(prp[:], lhsT=onescol[:], rhs=racc[n][:],
                         start=True, stop=True)
        prpe = spsum.tile([1, 256], F32, name="prpe", tag="sp")
        nc.tensor.matmul(prpe[:], lhsT=onescol[:], rhs=redge[n][:],
                         start=True, stop=True)
        prow = stpool.tile([1, 256], F32, name=f"prow_{n}")
        nc.vector.tensor_tensor(prow[:], prp[:], ivr_sb[:], op=ALU.mult)
        nc.vector.tensor_tensor(prow[:], prow[:], prpe[:], op=ALU.subtract)

        # conv: fpre[j] = sum_c wT[c, j] * pooled_sum[c]
        fpre = spsum.tile([72, 1], F32, name="fpre", tag="sp")
        for b in range(2):
            pcp = spsum.tile([128, 1], F32, name="pcp", tag="sp")
            nc.tensor.transpose(pcp[:], prow[0:1, b * 128:(b + 1) * 128],
                                idn[0:1, 0:1])
            pcol = stpool.tile([128, 1], F32, name=f"pcol_{n}_{b}")
            nc.scalar.copy(pcol[:], pcp[:])
            nc.tensor.matmul(fpre[:], lhsT=wtd_sb[:, b * 72:(b + 1) * 72],
                             rhs=pcol[:], start=(b == 0), stop=(b == 1))
        filt_sb = stpool.tile([72, 1], F32, name=f"filt_{n}")
        nc.scalar.activation(filt_sb[:], fpre[:], AF.Tanh,
                             bias=bnsb_sb[:, 1:2], scale=bnsb_sb[:, 0:1])
        # transpose [72,1] -> [1,72], then broadcast to [128,72]
        ftp = spsum.tile([1, 72], F32, name="ftp", tag="sp")
        nc.tensor.transpose(ftp[:], filt_sb[:], idn[0:72, 0:72])
        filt_row = stpool.tile([1, 72], F32, name=f"filtrow_{n}")
        nc.scalar.copy(filt_row[:], ftp[:])
        fbp = spsum.tile([128, 72], F32, name="fbp", tag="sp")
        nc.tensor.matmul(fbp[:], lhsT=ones_sb[:], rhs=filt_row[:],
                         start=True, stop=True)
        nc.scalar.copy(fbs[n][:], fbp[:])

        # beta row -> broadcast to B_n [128, 256]
        brow = stpool.tile([1, 256], F32, name=f"brow_{n}")
        nc.vector.tensor_tensor(brow[:], prow[:], mbrow_sb[:], op=ALU.mult)
        for b in range(2):
            bbp = spsum.tile([128, 128], F32, name="bbp", tag="sp")
            nc.tensor.matmul(bbp[:], lhsT=ones_sb[:],
                             rhs=brow[0:1, b * 128:(b + 1) * 128],
                             start=True, stop=True)
            nc.scalar.copy(b_n[n][:, b * 128:(b + 1) * 128], bbp[:])

    def g_build(n):
        # G_dx = f0*D_up + f1*I + f2*D_dn per (g, dx); reflect rows in D mats
        for g in range(G):
            for dx in range(3):
                blk = gt[n][:, (g * 3 + dx) * 128:(g * 3 + dx + 1) * 128]
                j0 = g * 9 + 0 * 3 + dx
                j1 = g * 9 + 1 * 3 + dx
                j2 = g * 9 + 2 * 3 + dx
                nc.scalar.activation(
                    blk, dmats_sb[:, 0:128], AF.Identity,
                    scale=fbs[n][:, j0:j0 + 1])
                nc.vector.scalar_tensor_tensor(
                    out=blk, in0=dmats_sb[:, 128:256],
                    scalar=fbs[n][:, j1:j1 + 1], in1=blk,
                    op0=ALU.mult, op1=ALU.add)
                nc.vector.scalar_tensor_tensor(
                    out=blk, in0=dmats_sb[:, 256:384],
                    scalar=fbs[n][:, j2:j2 + 1], in1=blk,
                    op0=ALU.mult, op1=ALU.add)

    def conv_seg(n, s):
        c0 = s * SEG_CH
        g = s  # segment == group
        seg = segs.pop((n, s))
        xs1 = xs1s.pop((n, s))
        s3 = seg.rearrange("p (c w) -> p c w", c=SEG_CH)
        xs13 = xs1.rearrange("p (c w) -> p c w", c=SEG_CH)
        outst = opool.tile([128, SEG_CH * W], BF16, name="outst")
        outst3 = outst.rearrange("p (c w) -> p c w", c=SEG_CH)
        nq = SEG_CH // BATCH_CH                           # 8 q-batches
        for w0 in range(0, nq, WAVE_Q):
            qs = list(range(w0, min(w0 + WAVE_Q, nq)))
            ps = {q: mpsum.tile([128, 512], F32, name="ps", tag="ps")
                  for q in qs}
            # identity on raw x first, then the three G taps (lhsT reused
            # across the wave's q batches -> few weight loads)
            for q in qs:
                nc.tensor.matmul(
                    ps[q][:], lhsT=idnb_sb[:],
                    rhs=s3[:, q * BATCH_CH:(q + 1) * BATCH_CH, 1:129],
                    start=True, stop=False)
            for dx in range(3):
                for q in qs:
                    nc.tensor.matmul(
                        ps[q][:],
                        lhsT=gt[n][:, (g * 3 + dx) * 128:(g * 3 + dx + 1) * 128],
                        rhs=xs13[:, q * BATCH_CH:(q + 1) * BATCH_CH, dx:dx + 128],
                        start=False, stop=(dx == 2))
            # evict: plain per-q copy, split DVE/ACT
            for q in qs:
                oq = outst[:, q * 512:(q + 1) * 512]
                if q % 3 == 2:
                    nc.vector.tensor_copy(oq, ps[q][:])
                else:
                    nc.scalar.copy(oq, ps[q][:])
        # in-place per-channel affine: out = s2*out + beta   (GPSIMD)
        for ci in range(SEG_CH):
            c = c0 + ci
            nc.gpsimd.tensor_scalar(
                out=outst3[:, ci, :], in0=outst3[:, ci, :],
                scalar1=sbc_sb[:, 256 + c:256 + c + 1],
                scalar2=b_n[n][:, c:c + 1],
                op0=ALU.mult, op1=ALU.add)
        nc.sync.dma_start(out_ap[n, s], outst[:])

    # ---- schedule ----
    for s in range(N_SEG):
        load_seg(0, s)
    filt_branch(0)
    g_build(0)
    for s in range(N_SEG):
        conv_seg(0, s)
        load_seg(1, s)
    filt_branch(1)
    g_build(1)
    for s in range(N_SEG):
        conv_seg(1, s)


def build_nc():
    nc = bacc.Bacc("TRN2", target_bir_lowering=False, debug=False)
    x_h = nc.dram_tensor("x", [N_PER_CORE, N_SEG, H, SEG_CH * WPAD], BF16,
                         kind="ExternalInput")
    dmats_h = nc.dram_tensor("dmats", [128, 384], F32, kind="ExternalInput")
    sbc_h = nc.dram_tensor("sbc", [128, 512], F32, kind="ExternalInput")
    ivr_h = nc.dram_tensor("ivr", [1, 256], F32, kind="ExternalInput")
    mbrow_h = nc.dram_tensor("mbrow", [1, 256], F32, kind="ExternalInput")
    wtd_h = nc.dram_tensor("wtd", [128, 144], F32, kind="ExternalInput")
    bnsb_h = nc.dram_tensor("bnsb", [72, 2], F32, kind="ExternalInput")
    idnb_h = nc.dram_tensor("idnb", [128, 128], BF16, kind="ExternalInput")
    out_h = nc.dram_tensor("out", [N_PER_CORE, N_SEG, H, SEG_CH * W], BF16,
                           kind="ExternalOutput")

    with tile.TileContext(nc) as tc:
        with ExitStack() as ctx:
            _build_kernel(ctx, tc, x_h.ap(), out_h.ap(), dmats_h.ap(),
                          sbc_h.ap(), ivr_h.ap(), mbrow_h.ap(), wtd_h.ap(),
                          bnsb_h.ap(), idnb_h.ap())
    nc.compile()
    return nc


def kernel(x, conv_w, bn_gamma, bn_beta, bn_mean, bn_var, lamb_l, lamb_h,
           inside_all, _trace=False, _trace_kwargs=None):
    x = np.ascontiguousarray(x, dtype=np.float32)
    consts = _host_consts(conv_w, bn_gamma, bn_beta, bn_mean, bn_var,
                          lamb_l, lamb_h, inside_all)
    xpacked = _host_pack_x(x)
    nc = build_nc()
    in_maps = []
    for i in range(N_CORES):
        m = {"x": np.ascontiguousarray(
            xpacked[i * N_PER_CORE:(i + 1) * N_PER_CORE])}
        m.update(consts)
        in_maps.append(m)
    kw = {}
    if _trace:
        kw["trace"] = True
        if _trace_kwargs:
            kw.update(_trace_kwargs)
    res = run_bass_kernel_spmd(nc, in_maps, list(range(N_CORES)), **kw)
    out = _host_unpack_out([res.results[i]["out"] for i in range(N_CORES)])
    if _trace:
        kernel.last_results = res
    return out
